# revision 6
# baseline (speedup 1.0000x reference)
"""Causal self-attention (S=2048, B=2, D=768, H=12) on 8 TRN2 NeuronCores.

Sharding: batch*heads across cores. Core c handles batch b = c//4 and the
3 heads hs = (c%4)*3 .. hs+2. Each core computes Q/K/V projections for its
heads, causal softmax(QK^T/sqrt(hd)) @ V, and its partial contribution to
the output projection y_part = att_cat @ wc_slice^T. The host gathers by
summing the 4 per-batch partials and adding the output bias.

Numerics: all matmuls in bf16 with fp32 PSUM accumulation. The causal mask is applied inside the scores accumulation
chain (maskneg^T @ ident adds -1e9 above the diagonal), the 1/sqrt(64)
score scale is folded into the EXP activation's scale parameter, and the
softmax denominator falls out of a ones-column in V. K's projection bias
is dropped entirely: it shifts every key's score for a given query by the
same amount, which softmax cancels.

Loop structure: one round per 128-row key block. Round r emits scores+EXP
for key block r of all 3 heads, and (lagged by 3 rounds) AV, normalize,
transpose, out-proj and the output DMA for query block r-3, so the tensor,
scalar, vector and DMA engines pipeline across rounds.
"""

import numpy as np
import ml_dtypes

import concourse.bass as bass
import concourse.mybir as mybir
import concourse.tile as tile
from concourse import bacc
from concourse.bass_utils import run_bass_kernel_spmd

S = 2048  # sequence length
B = 2     # batch
D = 768   # model dim
H = 12    # heads
HD = 64   # head dim
NCORES = 8
HPC = 3   # heads per core
DC = HPC * HD          # 192: per-core head dims
VW = HPC * (HD + 1)    # 195: V columns incl per-head ones column
NQB = S // 128         # 16 query/key blocks
LAG = 3                # rounds between scores(kb) and AV/out-proj(qi)
WS = 64.0              # fp8 weight prescale (host multiplies, drains divide)
F32 = mybir.dt.float32
BF16 = mybir.dt.bfloat16
F8 = mybir.dt.float8e4
BF = ml_dtypes.bfloat16
F8NP = ml_dtypes.float8_e4m3

TRACE = False          # set by test harness for profiled runs
LAST_RESULT = None     # BassKernelResults of the most recent run

_prog_cache = {}


def _build_program():
    nc = bacc.Bacc()
    AF = mybir.ActivationFunctionType
    DR = mybir.MatmulPerfMode.DoubleRow

    xtp = nc.declare_dram_parameter("xtp", [128, 6, S], BF16, isOutput=False)
    wqkp = nc.declare_dram_parameter("wqkp", [128, 6, 2 * DC], BF16, isOutput=False)
    wvp = nc.declare_dram_parameter("wvp", [128, 6, VW], BF16, isOutput=False)
    wvrow = nc.declare_dram_parameter("wvrow", [1, VW], BF16, isOutput=False)
    bq = nc.declare_dram_parameter("bq", [DC, 1], F32, isOutput=False)
    g = nc.declare_dram_parameter("g", [DC, D], BF16, isOutput=False)
    y = nc.declare_dram_parameter("y", [S, D], BF16, isOutput=True)

    with tile.TileContext(nc) as tc:
        with (
            tc.tile_pool(name="const", bufs=1) as constp,
            tc.tile_pool(name="acts", bufs=1) as actsp,
            tc.tile_pool(name="roll", bufs=2) as rollp,
            tc.tile_pool(name="small", bufs=4) as smallp,
            tc.tile_pool(name="sc", bufs=2, space="PSUM") as scp,
            tc.tile_pool(name="po", bufs=2, space="PSUM") as pop,
            tc.tile_pool(name="tp", bufs=2, space="PSUM") as tpp,
        ):
            # ---- constants / weights ----
            ones_row = constp.tile([1, S], BF16, tag="ones", name="ones")
            nc.vector.memset(ones_row[:], 1.0)
            ident = constp.tile([128, 128], BF16, tag="ident", name="ident")
            from concourse.masks import make_identity, make_causal_mask
            make_identity(nc, ident[:])
            identf = constp.tile([128, 128], F32, tag="identf", name="identf")
            make_identity(nc, identf[:])
            # maskneg[j, k] = -1e9 iff j < k; maskneg^T @ ident adds -1e9 to
            # score[k, q] where k > q (keys after the query).
            maskneg = constp.tile([128, 128], BF16, tag="mask", name="maskneg")
            make_causal_mask(nc, maskneg[:], mask_val=-1e9)

            xt_sb = constp.tile([128, 6, S], BF16, tag="xtp", name="xt_sb")
            nc.sync.dma_start(xt_sb[:], xtp[:])
            wqk_sb = constp.tile([128, 6, 2 * DC], BF16, tag="wqkp", name="wqk_sb")
            nc.sync.dma_start(wqk_sb[:], wqkp[:])
            wv_sb = constp.tile([128, 6, VW], BF16, tag="wvp", name="wv_sb")
            nc.sync.dma_start(wv_sb[:], wvp[:])
            wvr_sb = constp.tile([1, VW], BF16, tag="wvrow", name="wvr_sb")
            nc.sync.dma_start(wvr_sb[:], wvrow[:])
            bq_sb = []
            for h in range(HPC):
                t = constp.tile([64, 1], F32, tag=f"bq{h}", name=f"bq{h}")
                nc.sync.dma_start(t[:], bq[h * 64:(h + 1) * 64, :])
                bq_sb.append(t)
            g_sb = []
            for (p0, psz) in ((0, 128), (128, 64)):
                t = constp.tile([psz, D], BF16, tag=f"g{p0}", name=f"g{p0}")
                nc.sync.dma_start(t[:], g[p0:p0 + psz, :])
                g_sb.append(t)

            # ---- QK^T projection (fp8 DoubleRow, 256-deep per matmul) ----
            # column order of wqk: [q_h0 q_h1 q_h2 | k_h0 k_h1 k_h2], 64 each.
            qt = [constp.tile([64, S], BF16, tag=f"qt{h}", name=f"qt{h}")
                  for h in range(HPC)]
            kt = [constp.tile([64, S], BF16, tag=f"kt{h}", name=f"kt{h}")
                  for h in range(HPC)]
            for m in range(3):
                for n2 in range(2):
                    ps = scp.tile([128, 1024], F32, tag="sc", name="psqk")
                    for sub in range(2):
                        reg = ps[:, sub * 512:(sub + 1) * 512]
                        q0 = n2 * 1024 + sub * 512
                        for k in range(6):
                            nc.tensor.matmul(
                                reg, wqk_sb[:, k, m * 128:(m + 1) * 128],
                                xt_sb[:, k, q0:q0 + 512],
                                start=(k == 0), stop=(k == 5))
                    cols = slice(n2 * 1024, (n2 + 1) * 1024)
                    for half in range(2):
                        hh = 2 * m + half
                        src = ps[half * 64:(half + 1) * 64, :]
                        if hh < 3:
                            # q = psum + bq, on the (idle) scalar engine
                            nc.scalar.activation(
                                qt[hh][:, cols], src, AF.Identity,
                                bias=bq_sb[hh][:])
                        else:
                            nc.vector.tensor_copy(kt[hh - 3][:, cols], src)

            # ---- V projection (keys on partitions; ones col -> denom) ----
            v_sb = []
            for kb in range(NQB):
                ps = scp.tile([128, 1024], F32, tag="sc", name="psv")
                reg = ps[:, 0:VW]
                for k in range(6):
                    nc.tensor.matmul(
                        reg, xt_sb[:, k, kb * 128:(kb + 1) * 128],
                        wv_sb[:, k, :],
                        start=(k == 0), stop=False)
                nc.tensor.matmul(
                    reg, ones_row[:, kb * 128:(kb + 1) * 128], wvr_sb[:],
                    start=False, stop=True)
                t = actsp.tile([128, VW], BF16, tag=f"v{kb}", name=f"v{kb}")
                nc.vector.tensor_copy(t[:], reg)
                v_sb.append(t)

            # ---- pipelined rounds: scores(kb=r) + AV/out-proj(qi=r-LAG) ----
            # pt[h][kb][:, j] = exp((s[kb*128 + :, kb*128 + j])/8), exact
            # causal width, all tiles live until the final AV chain.
            pt = [[actsp.tile([128, S - kb * 128], BF16, tag=f"pt{h}_{kb}",
                              name=f"pt{h}_{kb}") for kb in range(NQB)]
                  for h in range(HPC)]

            for r in range(NQB + LAG):
                if r < NQB:
                    kb = r
                    W = S - kb * 128
                    for h in range(HPC):
                        for ci in range((W + 1023) // 1024):
                            w_c = min(1024, W - ci * 1024)
                            ps = scp.tile([128, 1024], F32, tag="sc", name="pss")
                            for sub in range((w_c + 511) // 512):
                                n = min(512, w_c - sub * 512)
                                qs = kb * 128 + ci * 1024 + sub * 512
                                diag = (ci == 0 and sub == 0)
                                nc.tensor.matmul(
                                    ps[:, sub * 512:sub * 512 + n],
                                    kt[h][:, kb * 128:(kb + 1) * 128],
                                    qt[h][:, qs:qs + n],
                                    start=True, stop=not diag)
                                if diag:
                                    nc.tensor.matmul(
                                        ps[:, 0:128], maskneg[:], ident[:],
                                        start=False, stop=True)
                            nc.scalar.activation(
                                pt[h][kb][:, ci * 1024:ci * 1024 + w_c],
                                ps[:, 0:w_c], AF.Exp, scale=0.125)
                if r >= LAG:
                    qi = r - LAG
                    att3 = rollp.tile([128, DC], F32, tag="att3", name="att3")
                    for h in range(HPC):
                        po = pop.tile([128, HD + 1], F32, tag="po", name="po")
                        for kb2 in range(qi + 1):
                            nc.tensor.matmul(
                                po[:],
                                pt[h][kb2][:, (qi - kb2) * 128:(qi - kb2 + 1) * 128],
                                v_sb[kb2][:, h * 65:h * 65 + 65],
                                start=(kb2 == 0), stop=(kb2 == qi))
                        rr = smallp.tile([128, 1], F32, tag="r", name="rr")
                        nc.vector.reciprocal(rr[:], po[:, HD:HD + 1])
                        nc.vector.tensor_scalar_mul(
                            att3[:, h * 64:(h + 1) * 64], po[:, 0:HD], rr[:])
                    # transpose att3 (head dims onto partitions), then y chunk
                    tA = tpp.tile([128, 512], F32, tag="tp", name="tA")
                    nc.tensor.transpose(tA[0:128, 0:128], att3[:, 0:128], identf[:])
                    nc.tensor.transpose(tA[0:64, 128:256], att3[:, 128:192], identf[:])
                    a0 = rollp.tile([128, 128], BF16, tag="attT0", name="a0")
                    a1 = rollp.tile([64, 128], BF16, tag="attT1", name="a1")
                    nc.vector.tensor_copy(a0[:], tA[0:128, 0:128])
                    nc.vector.tensor_copy(a1[:], tA[0:64, 128:256])
                    ys = rollp.tile([128, D], BF16, tag="ys", name="ys")
                    tB = tpp.tile([128, 512], F32, tag="tp", name="tB")
                    nc.tensor.matmul(tB[:], a0[:], g_sb[0][:, 0:512],
                                     start=True, stop=False)
                    nc.tensor.matmul(tB[:], a1[:], g_sb[1][:, 0:512],
                                     start=False, stop=True)
                    nc.vector.tensor_copy(ys[:, 0:512], tB[:])
                    tC = tpp.tile([128, 512], F32, tag="tp", name="tC")
                    nc.tensor.matmul(tC[:, 0:256], a0[:], g_sb[0][:, 512:768],
                                     start=True, stop=False)
                    nc.tensor.matmul(tC[:, 0:256], a1[:], g_sb[1][:, 512:768],
                                     start=False, stop=True)
                    nc.vector.tensor_copy(ys[:, 512:768], tC[:, 0:256])
                    nc.sync.dma_start(y[qi * 128:(qi + 1) * 128, :], ys[:])

    nc.finalize()
    return nc


def _pack_contraction(a):
    """[768, N] -> [128, 6, N]: row j -> (partition j%128, chunk j//128)."""
    n = a.shape[1]
    return np.ascontiguousarray(
        a.reshape(6, 128, n).transpose(1, 0, 2)).astype(BF)


def _prep_inputs(x, wq, bq, wk, bk, wv, bv, wc, bc):
    """Per-core input maps, all host-side slicing/transposition."""
    in_maps = []
    for c in range(NCORES):
        b = c // 4
        r0 = (c % 4) * HPC * HD
        rows = slice(r0, r0 + DC)
        xtb = np.ascontiguousarray(x[:, b, :].T)        # [768, 2048]
        wqk = np.concatenate([wq[rows], wk[rows]], axis=0).T  # [768, 384]
        wva = np.zeros((D, VW), np.float32)
        wvr = np.zeros((1, VW), np.float32)
        for j in range(HPC):
            hr = slice(r0 + j * HD, r0 + (j + 1) * HD)
            wva[:, j * 65:j * 65 + HD] = wv[hr].T
            wvr[0, j * 65:j * 65 + HD] = bv[hr]
            wvr[0, j * 65 + HD] = 1.0
        gm = np.ascontiguousarray(wc[:, rows].T).astype(BF)
        in_maps.append({
            "xtp": _pack_contraction(xtb),
            "wqkp": _pack_contraction(wqk),
            "wvp": _pack_contraction(wva),
            "wvrow": wvr.astype(BF),
            "bq": bq[rows][:, None].astype(np.float32),
            "g": gm,
        })
    return in_maps


def kernel(**inputs):
    global LAST_RESULT
    if "prog" not in _prog_cache:
        _prog_cache["prog"] = _build_program()
    nc = _prog_cache["prog"]

    args = {k: np.asarray(inputs[k], np.float32)
            for k in ("x", "wq", "bq", "wk", "bk", "wv", "bv", "wc", "bc")}
    in_maps = _prep_inputs(**args)
    res = run_bass_kernel_spmd(nc, in_maps, core_ids=list(range(NCORES)),
                               trace=TRACE)
    LAST_RESULT = res

    out = np.empty((S, B, D), np.float32)
    for b in range(B):
        acc = res.results[4 * b]["y"].astype(np.float32)
        for c in range(4 * b + 1, 4 * b + 4):
            acc = acc + res.results[c]["y"].astype(np.float32)
        out[:, b, :] = acc + args["bc"][None, :]
    return out


# revision 8
# speedup vs baseline: 1.2521x; 1.2521x over previous
"""Causal self-attention (S=2048, B=2, D=768, H=12) on 8 TRN2 NeuronCores.

Sharding: batch*heads across cores. Core c handles batch b = c//4 and the
3 heads hs = (c%4)*3 .. hs+2. Each core computes Q/K/V projections for its
heads, causal softmax(QK^T/sqrt(hd)) @ V, and its partial contribution to
the output projection y_part = att_cat @ wc_slice^T. The host gathers by
summing the 4 per-batch partials and adding the output bias.

Numerics: all matmuls bf16 with fp32 PSUM accumulation. The causal mask is
applied inside the scores accumulation chain (maskneg^T @ ident adds -1e9
above the diagonal), the 1/sqrt(64) score scale is folded into the EXP
activation's scale parameter, and the softmax denominator falls out of a
ones-column appended to V. K's projection bias is dropped: it shifts every
key's score for a given query equally, which softmax cancels.

Schedule: per-core wqk columns are grouped per head ([q_h | k_h] in each
128-col block) so head h's scores can start right after projection block h
drains - the scalar engine (EXP, the longest-pole engine with the PE) spins
up ~8us in. The main loop staggers heads by one round (scores(h, kb=r-h))
and lags AV/normalize by AVLAG rounds and transpose/out-proj/output-DMA one
round further, so the in-order tensor queue always has ready filler work
between EXP-paced score chunks. attT0 transposes ride the idle DMA XBAR.
"""

import numpy as np
import ml_dtypes

import concourse.bass as bass
import concourse.mybir as mybir
import concourse.tile as tile
from concourse import bacc
from concourse.bass_utils import run_bass_kernel_spmd

S = 2048  # sequence length
B = 2     # batch
D = 768   # model dim
H = 12    # heads
HD = 64   # head dim
NCORES = 8
HPC = 3   # heads per core
DC = HPC * HD          # 192: per-core head dims
VW = HPC * (HD + 1)    # 195: V columns incl per-head ones column
NQB = S // 128         # 16 query/key blocks
AVLAG = 5              # rounds between scores(kb) and AV/normalize(qi)
OPLAG = 6              # rounds between scores(kb) and out-proj(qj)
F32 = mybir.dt.float32
BF16 = mybir.dt.bfloat16
BF = ml_dtypes.bfloat16

TRACE = False          # set by test harness for profiled runs
LAST_RESULT = None     # BassKernelResults of the most recent run

_prog_cache = {}


def _build_program():
    nc = bacc.Bacc()
    AF = mybir.ActivationFunctionType

    xtp = nc.declare_dram_parameter("xtp", [128, 6, S], BF16, isOutput=False)
    wqkp = nc.declare_dram_parameter("wqkp", [128, 6, 2 * DC], BF16, isOutput=False)
    wvp = nc.declare_dram_parameter("wvp", [128, 6, VW], BF16, isOutput=False)
    wvrow = nc.declare_dram_parameter("wvrow", [1, VW], BF16, isOutput=False)
    bq = nc.declare_dram_parameter("bq", [DC, 1], F32, isOutput=False)
    g = nc.declare_dram_parameter("g", [DC, D], BF16, isOutput=False)
    y = nc.declare_dram_parameter("y", [S, D], BF16, isOutput=True)

    with tile.TileContext(nc) as tc:
        with (
            tc.tile_pool(name="const", bufs=1) as constp,
            tc.tile_pool(name="acts", bufs=1) as actsp,
            tc.tile_pool(name="roll", bufs=2) as rollp,
            tc.tile_pool(name="small", bufs=4) as smallp,
            tc.tile_pool(name="mm", bufs=5, space="PSUM") as mmp,
            tc.tile_pool(name="po", bufs=2, space="PSUM") as pop,
            tc.tile_pool(name="tr", bufs=1, space="PSUM") as trp,
        ):
            # ---- constants / weights ----
            ones_row = constp.tile([1, S], BF16, tag="ones", name="ones")
            nc.vector.memset(ones_row[:], 1.0)
            ident = constp.tile([128, 128], BF16, tag="ident", name="ident")
            from concourse.masks import make_identity, make_causal_mask
            make_identity(nc, ident[:])
            # maskneg[j, k] = -1e9 iff j < k; maskneg^T @ ident adds -1e9 to
            # score[k, q] where k > q (keys after the query).
            maskneg = constp.tile([128, 128], BF16, tag="mask", name="maskneg")
            make_causal_mask(nc, maskneg[:], mask_val=-1e9)

            xt_sb = constp.tile([128, 6, S], BF16, tag="xtp", name="xt_sb")
            nc.sync.dma_start(xt_sb[:], xtp[:])
            wqk_sb = constp.tile([128, 6, 2 * DC], BF16, tag="wqkp", name="wqk_sb")
            nc.sync.dma_start(wqk_sb[:], wqkp[:])
            wv_sb = constp.tile([128, 6, VW], BF16, tag="wvp", name="wv_sb")
            nc.sync.dma_start(wv_sb[:], wvp[:])
            wvr_sb = constp.tile([1, VW], BF16, tag="wvrow", name="wvr_sb")
            nc.sync.dma_start(wvr_sb[:], wvrow[:])
            bq_sb = []
            for h in range(HPC):
                t = constp.tile([64, 1], F32, tag=f"bq{h}", name=f"bq{h}")
                nc.sync.dma_start(t[:], bq[h * 64:(h + 1) * 64, :])
                bq_sb.append(t)
            g_sb = []
            for (p0, psz) in ((0, 128), (128, 64)):
                t = constp.tile([psz, D], BF16, tag=f"g{p0}", name=f"g{p0}")
                nc.sync.dma_start(t[:], g[p0:p0 + psz, :])
                g_sb.append(t)

            qt = [constp.tile([64, S], BF16, tag=f"qt{h}", name=f"qt{h}")
                  for h in range(HPC)]
            kt = [constp.tile([64, S], BF16, tag=f"kt{h}", name=f"kt{h}")
                  for h in range(HPC)]
            v_sb = [actsp.tile([128, VW], BF16, tag=f"v{kb}", name=f"v{kb}")
                    for kb in range(NQB)]
            # pt[h][kb][:, j] = exp(s[kb*128 + :, kb*128 + j]/8); exact causal
            # width, live until the last AV chain reads it.
            pt = [[actsp.tile([128, S - kb * 128], BF16, tag=f"pt{h}_{kb}",
                              name=f"pt{h}_{kb}") for kb in range(NQB)]
                  for h in range(HPC)]

            def proj_qk(h):
                # wqk col block h = [q_h (64) | k_h (64)]
                for n in range(4):
                    ps = mmp.tile([128, 512], F32, tag="mm", name="psqk")
                    for k in range(6):
                        nc.tensor.matmul(
                            ps[:], wqk_sb[:, k, h * 128:(h + 1) * 128],
                            xt_sb[:, k, n * 512:(n + 1) * 512],
                            start=(k == 0), stop=(k == 5))
                    cols = slice(n * 512, (n + 1) * 512)
                    nc.vector.tensor_scalar_add(
                        qt[h][:, cols], ps[0:64, :], bq_sb[h][:])
                    nc.vector.tensor_copy(kt[h][:, cols], ps[64:128, :])

            def proj_v(kb):
                ps = mmp.tile([128, 512], F32, tag="mm", name="psv")
                reg = ps[:, 0:VW]
                for k in range(6):
                    nc.tensor.matmul(
                        reg, xt_sb[:, k, kb * 128:(kb + 1) * 128],
                        wv_sb[:, k, :], start=(k == 0), stop=False)
                nc.tensor.matmul(
                    reg, ones_row[:, kb * 128:(kb + 1) * 128], wvr_sb[:],
                    start=False, stop=True)
                nc.vector.tensor_copy(v_sb[kb][:], reg)

            def scores(h, kb):
                W = S - kb * 128
                for ci in range((W + 511) // 512):
                    n = min(512, W - ci * 512)
                    qs = kb * 128 + ci * 512
                    ps = mmp.tile([128, 512], F32, tag="mm", name="pss")
                    nc.tensor.matmul(
                        ps[:, 0:n], kt[h][:, kb * 128:(kb + 1) * 128],
                        qt[h][:, qs:qs + n], start=True, stop=(ci != 0))
                    if ci == 0:
                        nc.tensor.matmul(
                            ps[:, 0:128], maskneg[:], ident[:],
                            start=False, stop=True)
                    nc.scalar.activation(
                        pt[h][kb][:, ci * 512:ci * 512 + n],
                        ps[:, 0:n], AF.Exp, scale=0.125)

            def av_block(qi):
                # one [128, 195] psum: 3 heads side by side; denom in col 64+65h
                po = pop.tile([128, VW], F32, tag="po", name="po")
                for h in range(HPC):
                    for kb2 in range(qi + 1):
                        nc.tensor.matmul(
                            po[:, h * 65:h * 65 + 65],
                            pt[h][kb2][:, (qi - kb2) * 128:(qi - kb2 + 1) * 128],
                            v_sb[kb2][:, h * 65:h * 65 + 65],
                            start=(kb2 == 0), stop=(kb2 == qi))
                rr = smallp.tile([128, HPC], F32, tag="r", name="rr")
                nc.vector.reciprocal(rr[:], po[:, 64::65])
                att3 = rollp.tile([128, DC], BF16, tag="att3", name="att3")
                for h in range(HPC):
                    nc.vector.tensor_scalar_mul(
                        att3[:, h * 64:(h + 1) * 64],
                        po[:, h * 65:h * 65 + 64], rr[:, h:h + 1])
                # attT0 (head dims 0..127) via the DMA XBAR; attT1 via the PE
                a0 = rollp.tile([128, 128], BF16, tag="attT0", name="a0")
                nc.sync.dma_start_transpose(a0[:], att3[:, 0:128])
                t1 = trp.tile([64, 128], BF16, tag="tr", name="t1")
                nc.tensor.transpose(t1[:], att3[:, 128:192], ident[:])
                a1 = rollp.tile([64, 128], BF16, tag="attT1", name="a1")
                nc.vector.tensor_copy(a1[:], t1[:])
                return a0, a1

            def outproj(qj, a0, a1):
                ys = rollp.tile([128, D], BF16, tag="ys", name="ys")
                for (n0, nsz) in ((0, 512), (512, 256)):
                    ps = mmp.tile([128, 512], F32, tag="mm", name="psy")
                    nc.tensor.matmul(ps[:, 0:nsz], a0[:], g_sb[0][:, n0:n0 + nsz],
                                     start=True, stop=False)
                    nc.tensor.matmul(ps[:, 0:nsz], a1[:], g_sb[1][:, n0:n0 + nsz],
                                     start=False, stop=True)
                    nc.vector.tensor_copy(ys[:, n0:n0 + nsz], ps[:, 0:nsz])
                nc.sync.dma_start(y[qj * 128:(qj + 1) * 128, :], ys[:])

            # ---- emission schedule ----
            proj_qk(0)
            proj_qk(1)
            attT = {}
            for r in range(NQB + 2 + OPLAG):
                # lagged AV / normalize / transpose first: always-ready filler
                # for the in-order tensor queue while EXP drains score chunks
                qi = r - AVLAG
                if 0 <= qi < NQB:
                    attT[qi] = av_block(qi)
                qj = r - OPLAG
                if 0 <= qj < NQB:
                    a0, a1 = attT.pop(qj)
                    outproj(qj, a0, a1)
                for h in range(HPC):
                    kb = r - h
                    if 0 <= kb < NQB:
                        scores(h, kb)
                if r == 0:
                    proj_qk(2)
                elif r in (1, 2):
                    for kb in range(8 * (r - 1), 8 * r):
                        proj_v(kb)

    nc.finalize()
    return nc


def _pack_contraction(a):
    """[768, N] -> [128, 6, N]: row j -> (partition j%128, chunk j//128)."""
    n = a.shape[1]
    return np.ascontiguousarray(
        a.reshape(6, 128, n).transpose(1, 0, 2)).astype(BF)


def _prep_inputs(x, wq, bq, wk, bk, wv, bv, wc, bc):
    """Per-core input maps, all host-side slicing/transposition."""
    in_maps = []
    for c in range(NCORES):
        b = c // 4
        r0 = (c % 4) * HPC * HD
        rows = slice(r0, r0 + DC)
        xtb = np.ascontiguousarray(x[:, b, :].T)        # [768, 2048]
        wqk = np.empty((D, 2 * DC), np.float32)         # [q_h|k_h] per block
        wva = np.zeros((D, VW), np.float32)
        wvr = np.zeros((1, VW), np.float32)
        for j in range(HPC):
            hr = slice(r0 + j * HD, r0 + (j + 1) * HD)
            wqk[:, j * 128:j * 128 + 64] = wq[hr].T
            wqk[:, j * 128 + 64:j * 128 + 128] = wk[hr].T
            wva[:, j * 65:j * 65 + HD] = wv[hr].T
            wvr[0, j * 65:j * 65 + HD] = bv[hr]
            wvr[0, j * 65 + HD] = 1.0
        gm = np.ascontiguousarray(wc[:, rows].T).astype(BF)
        in_maps.append({
            "xtp": _pack_contraction(xtb),
            "wqkp": _pack_contraction(wqk),
            "wvp": _pack_contraction(wva),
            "wvrow": wvr.astype(BF),
            "bq": bq[rows][:, None].astype(np.float32),
            "g": gm,
        })
    return in_maps


def kernel(**inputs):
    global LAST_RESULT
    if "prog" not in _prog_cache:
        _prog_cache["prog"] = _build_program()
    nc = _prog_cache["prog"]

    args = {k: np.asarray(inputs[k], np.float32)
            for k in ("x", "wq", "bq", "wk", "bk", "wv", "bv", "wc", "bc")}
    in_maps = _prep_inputs(**args)
    res = run_bass_kernel_spmd(nc, in_maps, core_ids=list(range(NCORES)),
                               trace=TRACE)
    LAST_RESULT = res

    out = np.empty((S, B, D), np.float32)
    for b in range(B):
        acc = res.results[4 * b]["y"].astype(np.float32)
        for c in range(4 * b + 1, 4 * b + 4):
            acc = acc + res.results[c]["y"].astype(np.float32)
        out[:, b, :] = acc + args["bc"][None, :]
    return out


# revision 9
# speedup vs baseline: 1.2730x; 1.0168x over previous
"""Causal self-attention (S=2048, B=2, D=768, H=12) on 8 TRN2 NeuronCores.

Sharding: batch*heads across cores. Core c handles batch b = c//4 and the
3 heads hs = (c%4)*3 .. hs+2. Each core computes Q/K/V projections for its
heads, causal softmax(QK^T/sqrt(hd)) @ V, and its partial contribution to
the output projection y_part = att_cat @ wc_slice^T. The host gathers by
summing the 4 per-batch partials and adding the output bias.

Numerics: all matmuls bf16 with fp32 PSUM accumulation. The causal mask is
applied inside the scores accumulation chain (maskneg^T @ ident adds -1e9
above the diagonal), the 1/sqrt(64) score scale is folded into the EXP
activation's scale parameter, and the softmax denominator falls out of a
ones-column appended to V. K's projection bias is dropped: it shifts every
key's score for a given query equally, which softmax cancels.

Schedule: per-core wqk columns are grouped per head ([q_h | k_h] in each
128-col block) so head h's scores can start right after projection block h
drains - the scalar engine (EXP, the longest-pole engine with the PE) spins
up ~8us in. The main loop staggers heads by one round (scores(h, kb=r-h))
and lags AV/normalize by AVLAG rounds and transpose/out-proj/output-DMA one
round further, so the in-order tensor queue always has ready filler work
between EXP-paced score chunks. attT0 transposes ride the idle DMA XBAR.
"""

import numpy as np
import ml_dtypes

import concourse.bass as bass
import concourse.mybir as mybir
import concourse.tile as tile
from concourse import bacc
from concourse.bass_utils import run_bass_kernel_spmd

S = 2048  # sequence length
B = 2     # batch
D = 768   # model dim
H = 12    # heads
HD = 64   # head dim
NCORES = 8
HPC = 3   # heads per core
DC = HPC * HD          # 192: per-core head dims
VW = HPC * (HD + 1)    # 195: V columns incl per-head ones column
NQB = S // 128         # 16 query/key blocks
AVLAG = 5              # rounds between scores(kb) and AV/normalize(qi)
OPLAG = 6              # rounds between scores(kb) and out-proj(qj)
F32 = mybir.dt.float32
BF16 = mybir.dt.bfloat16
BF = ml_dtypes.bfloat16

TRACE = False          # set by test harness for profiled runs
LAST_RESULT = None     # BassKernelResults of the most recent run

_prog_cache = {}


def _build_program():
    nc = bacc.Bacc()
    AF = mybir.ActivationFunctionType

    xtp = nc.declare_dram_parameter("xtp", [128, 6, S], BF16, isOutput=False)
    wqkp = nc.declare_dram_parameter("wqkp", [128, 6, 2 * DC], BF16, isOutput=False)
    wvp = nc.declare_dram_parameter("wvp", [128, 6, VW], BF16, isOutput=False)
    wvrow = nc.declare_dram_parameter("wvrow", [1, VW], BF16, isOutput=False)
    bq = nc.declare_dram_parameter("bq", [DC, 1], F32, isOutput=False)
    g = nc.declare_dram_parameter("g", [DC, D], BF16, isOutput=False)
    y = nc.declare_dram_parameter("y", [S, D], BF16, isOutput=True)

    with tile.TileContext(nc) as tc:
        with (
            tc.tile_pool(name="const", bufs=1) as constp,
            tc.tile_pool(name="acts", bufs=1) as actsp,
            tc.tile_pool(name="roll", bufs=2) as rollp,
            tc.tile_pool(name="small", bufs=4) as smallp,
            tc.tile_pool(name="mm", bufs=5, space="PSUM") as mmp,
            tc.tile_pool(name="po", bufs=2, space="PSUM") as pop,
            tc.tile_pool(name="tr", bufs=1, space="PSUM") as trp,
        ):
            # ---- constants / weights ----
            ident = constp.tile([128, 128], BF16, tag="ident", name="ident")
            from concourse.masks import make_identity, make_causal_mask
            make_identity(nc, ident[:])
            # maskneg[j, k] = -1e9 iff j < k; maskneg^T @ ident adds -1e9 to
            # score[k, q] where k > q (keys after the query).
            maskneg = constp.tile([128, 128], BF16, tag="mask", name="maskneg")
            make_causal_mask(nc, maskneg[:], mask_val=-1e9)

            xt_sb = constp.tile([128, 6, S], BF16, tag="xtp", name="xt_sb")
            wqk_sb = constp.tile([128, 6, 2 * DC], BF16, tag="wqkp", name="wqk_sb")
            wv_sb = constp.tile([128, 6, VW], BF16, tag="wvp", name="wv_sb")
            for k in range(6):
                nc.sync.dma_start(wqk_sb[:, k, :], wqkp[:, k, :])
                nc.sync.dma_start(xt_sb[:, k, 0:1024], xtp[:, k, 0:1024])
                nc.sync.dma_start(xt_sb[:, k, 1024:S], xtp[:, k, 1024:S])
                nc.sync.dma_start(wv_sb[:, k, :], wvp[:, k, :])
            wvr_sb = constp.tile([1, VW], BF16, tag="wvrow", name="wvr_sb")
            nc.sync.dma_start(wvr_sb[:], wvrow[:])
            wvr_bc = constp.tile([128, VW], BF16, tag="wvrbc", name="wvr_bc")
            nc.gpsimd.partition_broadcast(wvr_bc[:], wvr_sb[:])
            bq_sb = []
            for h in range(HPC):
                t = constp.tile([64, 1], F32, tag=f"bq{h}", name=f"bq{h}")
                nc.sync.dma_start(t[:], bq[h * 64:(h + 1) * 64, :])
                bq_sb.append(t)
            g_sb = []
            for (p0, psz) in ((0, 128), (128, 64)):
                t = constp.tile([psz, D], BF16, tag=f"g{p0}", name=f"g{p0}")
                nc.sync.dma_start(t[:], g[p0:p0 + psz, :])
                g_sb.append(t)

            qt = [constp.tile([64, S], BF16, tag=f"qt{h}", name=f"qt{h}")
                  for h in range(HPC)]
            kt = [constp.tile([64, S], BF16, tag=f"kt{h}", name=f"kt{h}")
                  for h in range(HPC)]
            v_sb = [actsp.tile([128, VW], BF16, tag=f"v{kb}", name=f"v{kb}")
                    for kb in range(NQB)]
            # pt[h][kb][:, j] = exp(s[kb*128 + :, kb*128 + j]/8); exact causal
            # width, live until the last AV chain reads it.
            pt = [[actsp.tile([128, S - kb * 128], BF16, tag=f"pt{h}_{kb}",
                              name=f"pt{h}_{kb}") for kb in range(NQB)]
                  for h in range(HPC)]

            def proj_qk(h):
                # wqk col block h = [q_h (64) | k_h (64)]
                for n in range(4):
                    ps = mmp.tile([128, 512], F32, tag="mm", name="psqk")
                    for k in range(6):
                        nc.tensor.matmul(
                            ps[:], wqk_sb[:, k, h * 128:(h + 1) * 128],
                            xt_sb[:, k, n * 512:(n + 1) * 512],
                            start=(k == 0), stop=(k == 5))
                    cols = slice(n * 512, (n + 1) * 512)
                    nc.vector.tensor_scalar_add(
                        qt[h][:, cols], ps[0:64, :], bq_sb[h][:])
                    nc.vector.tensor_copy(kt[h][:, cols], ps[64:128, :])

            def proj_v(kb):
                ps = mmp.tile([128, 512], F32, tag="mm", name="psv")
                reg = ps[:, 0:VW]
                for k in range(6):
                    nc.tensor.matmul(
                        reg, xt_sb[:, k, kb * 128:(kb + 1) * 128],
                        wv_sb[:, k, :], start=(k == 0), stop=(k == 5))
                nc.vector.tensor_add(v_sb[kb][:], reg, wvr_bc[:])

            def scores(h, kb):
                W = S - kb * 128
                for ci in range((W + 511) // 512):
                    n = min(512, W - ci * 512)
                    qs = kb * 128 + ci * 512
                    ps = mmp.tile([128, 512], F32, tag="mm", name="pss")
                    nc.tensor.matmul(
                        ps[:, 0:n], kt[h][:, kb * 128:(kb + 1) * 128],
                        qt[h][:, qs:qs + n], start=True, stop=(ci != 0))
                    if ci == 0:
                        nc.tensor.matmul(
                            ps[:, 0:128], maskneg[:], ident[:],
                            start=False, stop=True)
                    nc.scalar.activation(
                        pt[h][kb][:, ci * 512:ci * 512 + n],
                        ps[:, 0:n], AF.Exp, scale=0.125)

            def av_block(qi):
                # one [128, 195] psum: 3 heads side by side; denom in col 64+65h
                po = pop.tile([128, VW], F32, tag="po", name="po")
                for h in range(HPC):
                    for kb2 in range(qi + 1):
                        nc.tensor.matmul(
                            po[:, h * 65:h * 65 + 65],
                            pt[h][kb2][:, (qi - kb2) * 128:(qi - kb2 + 1) * 128],
                            v_sb[kb2][:, h * 65:h * 65 + 65],
                            start=(kb2 == 0), stop=(kb2 == qi))
                rr = smallp.tile([128, HPC], F32, tag="r", name="rr")
                nc.vector.reciprocal(rr[:], po[:, 64::65])
                att3 = rollp.tile([128, DC], BF16, tag="att3", name="att3")
                for h in range(HPC):
                    nc.vector.tensor_scalar_mul(
                        att3[:, h * 64:(h + 1) * 64],
                        po[:, h * 65:h * 65 + 64], rr[:, h:h + 1])
                # attT0 (head dims 0..127) via the DMA XBAR; attT1 via the PE
                a0 = rollp.tile([128, 128], BF16, tag="attT0", name="a0")
                nc.sync.dma_start_transpose(a0[:], att3[:, 0:128])
                t1 = trp.tile([64, 128], BF16, tag="tr", name="t1")
                nc.tensor.transpose(t1[:], att3[:, 128:192], ident[:])
                a1 = rollp.tile([64, 128], BF16, tag="attT1", name="a1")
                nc.vector.tensor_copy(a1[:], t1[:])
                return a0, a1

            def outproj(qj, a0, a1):
                ys = rollp.tile([128, D], BF16, tag="ys", name="ys")
                for (n0, nsz) in ((0, 512), (512, 256)):
                    ps = mmp.tile([128, 512], F32, tag="mm", name="psy")
                    nc.tensor.matmul(ps[:, 0:nsz], a0[:], g_sb[0][:, n0:n0 + nsz],
                                     start=True, stop=False)
                    nc.tensor.matmul(ps[:, 0:nsz], a1[:], g_sb[1][:, n0:n0 + nsz],
                                     start=False, stop=True)
                    nc.vector.tensor_copy(ys[:, n0:n0 + nsz], ps[:, 0:nsz])
                nc.sync.dma_start(y[qj * 128:(qj + 1) * 128, 0:512], ys[:, 0:512])
                nc.sync.dma_start(y[qj * 128:(qj + 1) * 128, 512:D], ys[:, 512:D])

            # ---- emission schedule ----
            proj_qk(0)
            proj_qk(1)
            attT = {}
            for r in range(NQB + 2 + OPLAG):
                # lagged AV / normalize / transpose first: always-ready filler
                # for the in-order tensor queue while EXP drains score chunks
                qi = r - AVLAG
                if 0 <= qi < NQB:
                    attT[qi] = av_block(qi)
                qj = r - OPLAG
                if 0 <= qj < NQB:
                    a0, a1 = attT.pop(qj)
                    outproj(qj, a0, a1)
                for h in range(HPC):
                    kb = r - h
                    if 0 <= kb < NQB:
                        scores(h, kb)
                if r == 0:
                    proj_qk(2)
                elif r in (1, 2):
                    for kb in range(8 * (r - 1), 8 * r):
                        proj_v(kb)

    nc.finalize()
    return nc


def _pack_contraction(a):
    """[768, N] -> [128, 6, N]: row j -> (partition j%128, chunk j//128)."""
    n = a.shape[1]
    return np.ascontiguousarray(
        a.reshape(6, 128, n).transpose(1, 0, 2)).astype(BF)


def _prep_inputs(x, wq, bq, wk, bk, wv, bv, wc, bc):
    """Per-core input maps, all host-side slicing/transposition."""
    in_maps = []
    for c in range(NCORES):
        b = c // 4
        r0 = (c % 4) * HPC * HD
        rows = slice(r0, r0 + DC)
        xtb = np.ascontiguousarray(x[:, b, :].T)        # [768, 2048]
        wqk = np.empty((D, 2 * DC), np.float32)         # [q_h|k_h] per block
        wva = np.zeros((D, VW), np.float32)
        wvr = np.zeros((1, VW), np.float32)
        for j in range(HPC):
            hr = slice(r0 + j * HD, r0 + (j + 1) * HD)
            wqk[:, j * 128:j * 128 + 64] = wq[hr].T
            wqk[:, j * 128 + 64:j * 128 + 128] = wk[hr].T
            wva[:, j * 65:j * 65 + HD] = wv[hr].T
            wvr[0, j * 65:j * 65 + HD] = bv[hr]
            wvr[0, j * 65 + HD] = 1.0
        gm = np.ascontiguousarray(wc[:, rows].T).astype(BF)
        in_maps.append({
            "xtp": _pack_contraction(xtb),
            "wqkp": _pack_contraction(wqk),
            "wvp": _pack_contraction(wva),
            "wvrow": wvr.astype(BF),
            "bq": bq[rows][:, None].astype(np.float32),
            "g": gm,
        })
    return in_maps


def kernel(**inputs):
    global LAST_RESULT
    if "prog" not in _prog_cache:
        _prog_cache["prog"] = _build_program()
    nc = _prog_cache["prog"]

    args = {k: np.asarray(inputs[k], np.float32)
            for k in ("x", "wq", "bq", "wk", "bk", "wv", "bv", "wc", "bc")}
    in_maps = _prep_inputs(**args)
    res = run_bass_kernel_spmd(nc, in_maps, core_ids=list(range(NCORES)),
                               trace=TRACE)
    LAST_RESULT = res

    out = np.empty((S, B, D), np.float32)
    for b in range(B):
        acc = res.results[4 * b]["y"].astype(np.float32)
        for c in range(4 * b + 1, 4 * b + 4):
            acc = acc + res.results[c]["y"].astype(np.float32)
        out[:, b, :] = acc + args["bc"][None, :]
    return out
